# revision 36
# baseline (speedup 1.0000x reference)
"""TRN2 Bass kernel for nn_AttentionModule (dense transformer attention block).

Reference computation (per sample b, x flattened to [256, 4096]):
    proj = conv_w @ x + conv_b                 [32, 4096]
    q    = (q_w @ proj + q_b).T                [4096, 32]
    k    = k_w @ proj + k_b                    [32, 4096]
    v    = v_w @ proj + v_b                    [256, 4096]
    attn = softmax(q @ k, axis=-1)             [4096(n), 4096(m)]
    out  = gamma * (v @ attn.T) + x            [256, 4096]

Sharding: 8 cores = 4 samples x 2 query-halves (2048 queries each). Each core
redundantly computes proj/k/v for its sample (cheap) and its half of the
queries. No cross-core communication. SPMD: odd cores receive x with the
spatial axis rolled by -2048 so "their" queries sit at columns 0:2048;
attention is permutation-invariant over keys so k/v column order is free.

On-core layout: scores are computed transposed, [m_keys(part), n_queries
(free)], so the exp'd scores chunks are directly usable as matmul weights
(lhsT) for the attn@V contraction over m, and the softmax denominator falls
out of the same matmul via an appended ones-column in the V^T projection
(column 256 of the [33,257] rhs; proj carries a ones-row 32 that also folds
in the v bias). No max-subtraction: exp'd scores are stored in bf16 (no
overflow below e^88); numerator and denominator share the same bf16 rounding
so softmax normalization cancels most of it.

The exp stream is split across TWO engines per score group: ACT does a true
exp on the first 1024 columns (m-chunks 4g,4g+1), the DVE does a Schraudolph
bit-trick exp on the last 1024 (m-chunks 4g+2,4g+3): bits = round(s*128/ln2
+ B) as uint16, bitcast to bf16 == 2^(s/ln2) with ~3% max rel error. Softmax
normalization cancels the common-mode part; the end-to-end output error from
this is ~1e-3 (gamma ~0.1 further damps it). This halves the softmax-exp
wall (the single-ACT version serialized ~55us of exp behind the PE).

PSUM budget (8 banks): one 4-bank slot for score-group / prologue 2048-wide
tiles (tag "ps"), two 2-bank slots (tag "po") for attnout accumulators and
all 1024-wide prologue tiles. Per attnout block the emission order is
[SG(2nb) | att 0:16 | SG(2nb+1) | att 16:32 | epilogue] so every PSUM drain
(1.2us exp / 0.7us epilogue) is covered by attnout streaming.

Residual xT and output travel as fp16 with [128, 16, 256] DRAM layout (8KB /
2KB contiguous lines); the host does the transposes. gamma is folded into
v_w/v_b host-side. fp16 feeds the q/k score path.
"""

import numpy as np
from contextlib import ExitStack

import concourse.bass as bass
import concourse.bacc as bacc
import concourse.tile as tile
from concourse import mybir
from concourse.bass_utils import run_bass_kernel_spmd

F32 = mybir.dt.float32
F16 = mybir.dt.float16
BF16 = mybir.dt.bfloat16
U16 = mybir.dt.uint16

B, C, H, W = 4, 256, 64, 64
HW = H * W          # 4096 keys (m)
NQ = HW // 2        # 2048 queries per core (n)
C8 = 32             # qk head dim (e) / proj channels (d)
NSUP = 512          # queries per attention super-block
NBLK = 128          # queries per attnout block
MCH = 128           # keys per m-chunk (one lhsT tile)
N_MCH = HW // MCH   # 32 m-chunks
VN = C + 1          # 257: v channels + ones column (softmax denominator)
NBT = NQ // NBLK    # 16 attnout blocks total

# Schraudolph exp-in-bf16-bits: bits = round(s * 128/ln2 + SCH_B); bitcast
# bf16 ~= e^s (max rel err ~3%). Valid (bits in (0, 32768)) for |s| < 88.
SCH_A = 184.66509904026207
SCH_B = 16250.49

_CACHED = {}


def build_nc():
    nc = bacc.Bacc("TRN2", target_bir_lowering=False, debug=False)
    d_x16 = nc.dram_tensor("x16", [C, HW], F16, kind="ExternalInput").ap()
    d_xT = nc.dram_tensor("xT", [128, NBT, C], F16, kind="ExternalInput").ap()
    d_cwT = nc.dram_tensor("cwT", [2, 128, C8], F16, kind="ExternalInput").ap()
    d_cb = nc.dram_tensor("cb", [C8, 1], F32, kind="ExternalInput").ap()
    # k/q weights carry their bias as row 32, contracted against proj's
    # ones-row — no separate bias op needed.
    d_kwT = nc.dram_tensor("kwT", [C8 + 1, C8], F16, kind="ExternalInput").ap()
    d_qwT = nc.dram_tensor("qwT", [C8 + 1, C8], F16, kind="ExternalInput").ap()
    d_vwb = nc.dram_tensor("vwb", [C8 + 1, VN], F16, kind="ExternalInput").ap()
    d_outT = nc.dram_tensor("outT", [128, NBT, C], F16, kind="ExternalOutput").ap()

    IDENT = mybir.ActivationFunctionType.Identity
    EXP = mybir.ActivationFunctionType.Exp
    MUL = mybir.AluOpType.mult
    ADD = mybir.AluOpType.add

    with tile.TileContext(nc) as tc, ExitStack() as ctx:
        const_pool = ctx.enter_context(tc.tile_pool(name="const", bufs=1))
        big_pool = ctx.enter_context(tc.tile_pool(name="big", bufs=1))

        # ---- constants / inputs ----
        cwT = const_pool.tile([128, 2, C8], F16)
        kwT = const_pool.tile([C8 + 1, C8], F16)
        qwT = const_pool.tile([C8 + 1, C8], F16)
        vwb = const_pool.tile([C8 + 1, VN], F16)
        cb = const_pool.tile([C8, 1], F32)
        warm = const_pool.tile([128, 512], F16)
        for a in range(2):
            nc.sync.dma_start(cwT[:, a, :], d_cwT[a])
        nc.sync.dma_start(cb[:], d_cb)
        # later-needed weights ride the gpsimd queue ahead of the x16 tail
        nc.gpsimd.dma_start(kwT[:], d_kwT)
        nc.gpsimd.dma_start(qwT[:], d_qwT)
        nc.gpsimd.dma_start(vwb[:], d_vwb)
        nc.gpsimd.memset(warm[:], 0.0)

        # x16: two c-halves [128, HW] fp16 (matmul operand); 2048-col chunks
        # (4KB contiguous lines) across FOUR queues — the two HWDGE rings
        # carry the critical cols 0:2048 of each half (all the first proj
        # slice needs), the SWDGE queues carry the back half.
        x16 = [big_pool.tile([128, HW], F16, tag=f"x16_{i}", name=f"x16_{i}")
               for i in range(2)]
        d_x16v = d_x16.rearrange("(a p) m -> a p m", p=128)
        # the first proj sub-slice needs cols 0:512 of BOTH halves — those
        # two small sub-chunks lead the two HWDGE queues so the PE unblocks
        # at the earliest possible moment.
        for j in range(4):
            sl = bass.ts(j, 512)
            eng = nc.sync if j % 2 == 0 else nc.scalar
            oth = nc.scalar if j % 2 == 0 else nc.sync
            eng.dma_start(x16[0][:, sl], d_x16v[0][:, sl])
            oth.dma_start(x16[1][:, sl], d_x16v[1][:, sl])
        c1a, c1b = bass.ds(2048, 1024), bass.ds(3072, 1024)
        nc.sync.dma_start(x16[0][:, c1a], d_x16v[0][:, c1a])
        nc.scalar.dma_start(x16[1][:, c1a], d_x16v[1][:, c1a])
        nc.sync.dma_start(x16[1][:, c1b], d_x16v[1][:, c1b])
        nc.scalar.dma_start(x16[0][:, c1b], d_x16v[0][:, c1b])

        # xT: residual input, [128, nb, 256] fp16, one DMA with 8KB lines.
        # Queued on sync BEHIND the x16 chunk so it doesn't steal SDMA
        # bandwidth from the critical-path input; needed only at the first
        # epilogue (~35us in).
        xT = big_pool.tile([128, NBT, C], F16)
        nc.sync.dma_start(xT[:], d_xT)

        proj = big_pool.tile([C8 + 1, HW], F16)   # row 32 = ones
        nc.gpsimd.memset(proj[C8 : C8 + 1, :], 1.0)
        k4 = big_pool.tile([128, HW], F16)        # k replicated on 4 row-groups
        qT4 = big_pool.tile([128, NQ], F16)       # query half, replicated x4
        vt = big_pool.tile([128, N_MCH * VN], BF16)  # vT' chunks [m=128, 257]
        HGRP = 1024                               # e-tile cols per score group

        # ---- PSUM: 8 banks as 2 double-buffered tags ----
        # pslo/pshi: the two halves of a score group, separate tiles so the
        # ACT exp and DVE schrau drains are dependency-independent (a shared
        # PSUM tile serializes its cross-engine readers). bufs=2 so the next
        # group's matmuls never wait on the previous group's drain — that
        # serial chain was the prologue wall. Attnout accumulators and the
        # prologue/v-build tiles ride the same two tags.
        psum = ctx.enter_context(tc.tile_pool(name="psum", bufs=1,
                                              space="PSUM"))
        att_pool = ctx.enter_context(tc.tile_pool(name="att", bufs=2))
        out_pool = ctx.enter_context(tc.tile_pool(name="outp", bufs=3))

        def ps_tile(tag, shape, name):
            return psum.tile(shape, F32, tag=tag,
                             bufs=2 if tag == "pslo" else 1, name=name)

        def po_tile(name):
            return psum.tile([128, 512], F32, tag="po", bufs=2, name=name)

        # PE warmup: dummy matmuls on zeros while the input DMAs land, so
        # the HAM clock-gate is released before the real work starts.
        # PE warmup: dummy matmuls on zeros while the input DMAs land, so
        # the HAM clock-gate is released before the real work starts.
        pw = po_tile("pw")
        for _ in range(12):
            nc.tensor.matmul(pw[0:C8, :], cwT[:, 0, :], warm[:])

        # proj = conv_w @ x + conv_b (K=256 over 2 chunks); bias applied by
        # ACT on the low 1024 columns, DVE on the high 1024 of each slice.
        def emit_proj_slice(s, tags):
            for h in range(2):
                pp = ps_tile(tags[h], [C8, 1024], f"pp{s}{h}")
                for jj in range(2):
                    sl = bass.ts(jj, 512)
                    gsl = bass.ds(s * 2048 + h * 1024 + jj * 512, 512)
                    nc.tensor.matmul(pp[:, sl], cwT[:, 0, :], x16[0][:, gsl],
                                     start=True, stop=False)
                    nc.tensor.matmul(pp[:, sl], cwT[:, 1, :], x16[1][:, gsl],
                                     start=False, stop=True)
                dst = proj[0:C8, bass.ds(s * 2048 + h * 1024, 1024)]
                if h == 0:
                    nc.scalar.activation(dst, pp[:], IDENT, bias=cb[:])
                else:
                    nc.vector.tensor_scalar(dst, pp[:], cb[:], None, ADD)

        # qT4 = q_w' @ proj' (bias via proj ones-row), x4 col-groups
        def emit_q(tags):
            for h in range(2):
                pq = ps_tile(tags[h], [128, 1024], f"pq{h}")
                for jj in range(2):
                    sl = bass.ts(jj, 512)
                    psl = bass.ds(h * 1024 + jj * 512, 512)
                    for g in range(4):
                        nc.tensor.matmul(pq[bass.ts(g, 32), sl], qwT[:],
                                         proj[:, psl], tile_position=(0, 32 * g))
                dst = qT4[:, bass.ds(h * 1024, 1024)]
                if h == 0:
                    nc.scalar.copy(dst, pq[:])
                else:
                    nc.vector.tensor_copy(dst, pq[:])

        # k4 = k_w' @ proj' on all 4 col-groups (x4 replication)
        def emit_k_slice(s, tags):
            for h in range(2):
                pk = ps_tile(tags[h], [128, 1024], f"pk{s}{h}")
                for jj in range(2):
                    sl = bass.ts(jj, 512)
                    gsl = bass.ds(s * 2048 + h * 1024 + jj * 512, 512)
                    for g in range(4):
                        nc.tensor.matmul(pk[bass.ts(g, 32), sl], kwT[:],
                                         proj[:, gsl], tile_position=(0, 32 * g))
                dst = k4[:, bass.ds(s * 2048 + h * 1024, 1024)]
                if h == 0:
                    nc.scalar.copy(dst, pk[:])
                else:
                    nc.vector.tensor_copy(dst, pk[:])

        # ---- attention ----
        n_sup = NQ // NSUP                # 4 super-blocks of 512 queries
        n_blk = NSUP // NBLK              # 4 attnout blocks per super
        GCH = 4                           # m-chunks per scores group
        n_grp = N_MCH // GCH              # 8 scores groups per super
        # exp'd scores live in TWO tiles per super: e_lo (chunks 4g,4g+1,
        # ACT-written true exp) and e_hi (chunks 4g+2,4g+3, DVE-written
        # Schraudolph bits via a uint16 bitcast AP). Separate tiles keep the
        # two engines' writes dependency-independent — a bitcast AP defeats
        # subtile tracking, and a shared tile would serialize every DVE op
        # WAW behind the preceding ACT op.
        e_los, e_his = {}, {}

        def alloc_e(ns):
            e_los[ns] = att_pool.tile([128, n_grp * HGRP], BF16, tag="e_lo",
                                      name=f"e_lo_{ns}")
            e_his[ns] = att_pool.tile([128, n_grp * HGRP], BF16, tag="e_hi",
                                      name=f"e_hi_{ns}")

        def e_chunk(ns, mi, nb):
            g, r = mi // GCH, mi % GCH
            t = e_los[ns] if r < 2 else e_his[ns]
            return t[:, bass.ds(g * HGRP + (r % 2) * NSUP + nb * NBLK, NBLK)]

        def emit_score_group(ns, g):
            nsl = bass.ts(ns, NSUP)
            ps_lo = ps_tile("pslo", [128, 1024], f"psl_{ns}_{g}")
            ps_hi = ps_tile("pshi", [128, 1024], f"psh_{ns}_{g}")
            for i in range(GCH):
                mi = GCH * g + i
                ps = ps_lo if i < 2 else ps_hi
                nc.tensor.matmul(
                    ps[:, bass.ts(i % 2, NSUP)],
                    k4[bass.ts(i, 32), bass.ts(mi, MCH)],
                    qT4[bass.ts(i, 32), nsl],
                    tile_position=(32 * i, 0),
                )
            nc.scalar.activation(e_los[ns][:, bass.ts(g, HGRP)],
                                 ps_lo[:], EXP)
            nc.vector.tensor_scalar(
                e_his[ns][:, bass.ts(g, HGRP)].bitcast(U16),
                ps_hi[:], SCH_A, SCH_B, MUL, ADD)

        # scores + exp for super 0 run interleaved with the vT' build; vt
        # copies alternate between ACT and DVE so both engine streams stay
        # balanced with the split exp.
        def emit_v_pair(vg):        # vg in 0..15, chunks 2vg, 2vg+1
            pv = ps_tile("pslo", [128, 2, 512], f"pv{vg}")
            for i in range(2):
                mi = 2 * vg + i
                nc.tensor.matmul(pv[:, i, 0:VN], proj[:, bass.ts(mi, MCH)],
                                 vwb[:])
            vt_sl = vt[:, bass.ds(2 * vg * VN, 2 * VN)].rearrange(
                "p (a v) -> p a v", v=VN)
            if vg % 2 == 0:
                nc.scalar.copy(vt_sl, pv[:, :, 0:VN])
            else:
                nc.vector.tensor_copy(vt_sl, pv[:, :, 0:VN])

        osb_cur = [None]

        def emit_block_epilogue(po, nbg):
            rcol = out_pool.tile([128, 1], F32, tag="rcol",
                                 name=f"rcol_{nbg}")
            nc.vector.reciprocal(rcol[:], po[:, C : C + 1])
            anorm = out_pool.tile([128, C], F32, tag="anorm",
                                  name=f"anorm_{nbg}")
            # normalization multiply on ACT (scale is a per-partition AP) to
            # balance the DVE, which carries the schrau + recip + add stream
            nc.scalar.activation(anorm[:], po[:, 0:C], IDENT, scale=rcol[:])
            if nbg >= 12:
                # last super: per-block DMA so the final transfer is small
                osb = out_pool.tile([128, 1, C], F16, tag="osb1",
                                    name=f"osb_{nbg}")
                nc.vector.tensor_add(osb[:, 0, :], anorm[:], xT[:, nbg, :])
                nc.sync.dma_start(d_outT[:, nbg : nbg + 1, :], osb[:])
                return
            if nbg % 2 == 0:
                osb_cur[0] = out_pool.tile([128, 2, C], F16, tag="osb",
                                           name=f"osb_{nbg}")
            osb = osb_cur[0]
            nc.vector.tensor_add(osb[:, nbg % 2, :], anorm[:], xT[:, nbg, :])
            if nbg % 2 == 1:
                nc.sync.dma_start(d_outT[:, nbg - 1 : nbg + 1, :], osb[:])

        # ---- prologue ----
        # Order is latency-critical: score groups 0-3 need only the FIRST
        # 2048 columns of proj/k4 (slice 0), so the super-0 exp stream —
        # the long pole before attnout can run — starts before the second
        # x16 half has even landed.
        emit_proj_slice(0, ("pslo", "pshi"))
        emit_q(("pslo", "pshi"))
        emit_k_slice(0, ("pslo", "pshi"))
        alloc_e(0)
        # ALL score groups first, back-to-back, so the exp/schrau streams
        # run at full engine cadence — e[0] is the critical path to the
        # first attnout block. The whole vT' build comes after; its copies
        # queue behind the exps and are consumed just-in-time by block 0.
        for g in range(n_grp // 2):
            emit_score_group(0, g)
        emit_proj_slice(1, ("pslo", "pshi"))
        emit_k_slice(1, ("pslo", "pshi"))
        for g in range(n_grp // 2, n_grp):
            emit_score_group(0, g)
        for vg in range(2 * n_grp):
            emit_v_pair(vg)

        # ---- steady state ----
        # Per block: [SG(ns+1, 2nb) | att 0:16 | SG(ns+1, 2nb+1) | att 16:32
        # | epilogue]; every PSUM drain is covered by attnout streaming.
        def emit_att_half(po, ns, nb, lo):
            for mi in range(lo, lo + N_MCH // 2):
                nc.tensor.matmul(
                    po[:, 0:VN], e_chunk(ns, mi, nb), vt[:, bass.ts(mi, VN)],
                    start=(mi == 0), stop=(mi == N_MCH - 1),
                )

        for ns in range(n_sup):
            if ns + 1 < n_sup:
                alloc_e(ns + 1)
            for nb in range(n_blk):
                po = po_tile(f"po_{ns}_{nb}")
                if ns + 1 < n_sup:
                    emit_score_group(ns + 1, 2 * nb)
                emit_att_half(po, ns, nb, 0)
                if ns + 1 < n_sup:
                    emit_score_group(ns + 1, 2 * nb + 1)
                emit_att_half(po, ns, nb, N_MCH // 2)
                emit_block_epilogue(po, ns * n_blk + nb)
            e_los.pop(ns)
            e_his.pop(ns)

    nc.compile()
    return nc


def _prep_in_maps(x, conv_w, conv_b, q_w, q_b, k_w, k_b, v_w, v_b, gamma):
    g = np.float32(gamma[0])
    cwT = np.ascontiguousarray(conv_w.T.reshape(2, 128, C8)).astype(np.float16)
    kwT = np.concatenate([k_w.T, k_b[None, :]], axis=0).astype(np.float16)
    qwT = np.concatenate([q_w.T, q_b[None, :]], axis=0).astype(np.float16)
    vwb = np.zeros((C8 + 1, VN), np.float16)
    vwb[0:C8, 0:C] = (g * v_w).T.astype(np.float16)
    vwb[C8, 0:C] = (g * v_b).astype(np.float16)
    vwb[C8, C] = 1.0
    cb = conv_b.reshape(C8, 1).astype(np.float32)

    in_maps = []
    for core in range(8):
        b, hf = core // 2, core % 2
        xf = np.asarray(x[b], np.float32).reshape(C, HW)
        if hf:
            # rotate spatial columns: this core's query half -> cols 0:2048
            xf = np.roll(xf, -NQ, axis=1)
        xTh = np.ascontiguousarray(
            xf[:, 0:NQ].T.reshape(NBT, 128, C).transpose(1, 0, 2)
        ).astype(np.float16)
        in_maps.append({
            "x16": np.ascontiguousarray(xf).astype(np.float16),
            "xT": xTh,
            "cwT": cwT, "cb": cb, "kwT": kwT, "qwT": qwT, "vwb": vwb,
        })
    return in_maps


def kernel(x, conv_w, conv_b, q_w, q_b, k_w, k_b, v_w, v_b, gamma, **run_kw):
    if "nc" not in _CACHED:
        _CACHED["nc"] = build_nc()
    nc = _CACHED["nc"]
    in_maps = _prep_in_maps(x, conv_w, conv_b, q_w, q_b, k_w, k_b, v_w, v_b,
                            gamma)
    res = run_bass_kernel_spmd(nc, in_maps, core_ids=list(range(8)), **run_kw)
    _CACHED["last_result"] = res
    out = np.empty((B, C, HW), np.float32)
    for core in range(8):
        b, hf = core // 2, core % 2
        oc = np.asarray(res.results[core]["outT"])      # [128, 16, 256] fp16
        ocf = oc.astype(np.float32).transpose(1, 0, 2).reshape(NQ, C)
        out[b, :, hf * NQ : (hf + 1) * NQ] = ocf.T
    return out.reshape(B, C, H, W)
